# revision 22
# baseline (speedup 1.0000x reference)
"""BayesianLinear forward on 8 Trainium2 NeuronCores.

y = x @ W^T + b with W = w_mu + softplus(w_rho) * eps_w,
                     b = b_mu + softplus(b_rho) * eps_b.

Sharding: column-parallel (output features / 8). Each core samples its
weight shard on-chip and computes y^T[o_shard, :] = W_shard @ x^T.

Inputs stream in bf16 (the 2e-2 tolerance leaves ~10x margin): this
halves HBM traffic vs fp32 (44 MiB/core vs 88), which is what made the
fp32 version DMA-bound (91% DMA busy, PE starved and HAM-throttled).
Here the PE is the bottleneck: 1024 back-to-back 128x128x512 matmuls
(~213 ns each warm) ~= 219 us/core.

Structure: the whole sampled W shard stays resident in SBUF (32 KiB/
partition bf16) and each (token-tile, out-subtile) accumulates its full
k=4096 contraction directly in one PSUM bank (no SBUF accumulators).
Phase A pipelines param-DMA -> softplus sampling -> matmuls across token
tiles 0-1 (8 open PSUM groups) so the PE starts ~4 us in while params
stream; phase B runs token tiles 2-7 k-dense with x double-buffered.
"""

import numpy as np

# Problem shape (hardcoded per contest rules; kernel.py must be self-contained).
IN_F = 4096
OUT_F = 4096
N_TOK = 4096
N_CORES = 8
O_SHARD = OUT_F // N_CORES  # 512 output features per core

P = 128                     # SBUF partitions
KT = IN_F // P              # 32 contraction tiles
MS = O_SHARD // P           # 4 output-feature subtiles per core
N_TILE = 512                # moving-operand tile (fp32 PSUM bank limit)
NT = N_TOK // N_TILE        # 8 token tiles

# Param/sampling chunk sizes (k-tiles per chunk) for the phase-A pipeline.
# Uniform 4-ktile chunks: a 2-ktile chunk is only ~3.4us of PE work, not
# enough for the two-chunk-ahead prefetch to cover the ~10us param
# DMA+sampling latency.
CHUNKS = [2, 4, 4, 4, 4, 4, 4, 4, 2]
CHMAX = max(CHUNKS)
assert sum(CHUNKS) == KT

_CACHE = {}


def _pin_act_table(bacc, mybir):
    """Keep Exp and Ln only in the one ACT table that has both, so the
    compiler never inserts per-op table reloads (1.3 us each, and they sit
    on the weight-sampling critical path)."""
    if getattr(bacc.get_activation_tables, "_pinned", False):
        return
    orig = bacc.get_activation_tables
    EXP = mybir.ActivationFunctionType.Exp
    LN = mybir.ActivationFunctionType.Ln

    def pinned(arch):
        tables = orig(arch)
        for name, funcs in tables.items():
            if name != "natural_log_exp_and_others":
                funcs.discard(EXP)
                funcs.discard(LN)
        return tables

    pinned._pinned = True
    bacc.get_activation_tables = pinned


def _build_nc():
    import concourse.bass as bass  # noqa: F401
    from concourse import bacc, mybir
    from concourse.tile import TileContext

    _pin_act_table(bacc, mybir)

    f32 = mybir.dt.float32
    f32r = mybir.dt.float32r
    bf16 = mybir.dt.bfloat16
    AF = mybir.ActivationFunctionType

    nc = bacc.Bacc("TRN2", target_bir_lowering=False, debug=False,
                   num_devices=N_CORES)

    x_t = nc.dram_tensor("x_t", [IN_F, N_TOK], bf16, kind="ExternalInput")
    w_mu_t = nc.dram_tensor("w_mu_t", [IN_F, O_SHARD], bf16, kind="ExternalInput")
    w_rho_t = nc.dram_tensor("w_rho_t", [IN_F, O_SHARD], bf16, kind="ExternalInput")
    eps_w_t = nc.dram_tensor("eps_w_t", [IN_F, O_SHARD], bf16, kind="ExternalInput")
    b_mu = nc.dram_tensor("b_mu", [O_SHARD], f32, kind="ExternalInput")
    b_rho = nc.dram_tensor("b_rho", [O_SHARD], f32, kind="ExternalInput")
    eps_b = nc.dram_tensor("eps_b", [O_SHARD], f32, kind="ExternalInput")
    y_t = nc.dram_tensor("y_t", [O_SHARD, N_TOK], f32, kind="ExternalOutput")

    chunk_start = [sum(CHUNKS[:c]) for c in range(len(CHUNKS))]

    with TileContext(nc) as tc:
        with (
            tc.tile_pool(name="wpool", bufs=1) as wpool,
            tc.tile_pool(name="ppool", bufs=3) as ppool,
            tc.tile_pool(name="spool", bufs=2) as spool,
            tc.tile_pool(name="bpool", bufs=1) as bpool,
            tc.tile_pool(name="xpool", bufs=3) as xpool,
            tc.tile_pool(name="opool", bufs=3) as opool,
            tc.tile_pool(name="psum", bufs=8, space="PSUM") as psum,
        ):
            # ---- bias vector: b = b_mu + softplus(b_rho) * eps_b ----
            # laid out [P, MS]: partition p of output subtile ms holds
            # b[ms*128 + p].
            bmu_sb = bpool.tile([P, MS], f32, tag="bmu")
            brho_sb = bpool.tile([P, MS], f32, tag="brho")
            beps_sb = bpool.tile([P, MS], f32, tag="beps")
            bvec = bpool.tile([P, MS], f32, tag="bvec")

            def compute_bias():
                nc.scalar.dma_start(bmu_sb[:],
                                    b_mu.rearrange("(s p) -> p s", p=P))
                nc.scalar.dma_start(brho_sb[:],
                                    b_rho.rearrange("(s p) -> p s", p=P))
                nc.scalar.dma_start(beps_sb[:],
                                    eps_b.rearrange("(s p) -> p s", p=P))
                # softplus(r) = ln(1 + exp(r)); Exp/Ln share one ACT table
                # and the Ln's +1 folds into the activation bias operand.
                nc.scalar.activation(bvec[:], brho_sb[:], AF.Exp)
                nc.scalar.activation(bvec[:], bvec[:], AF.Ln, bias=1.0)
                nc.vector.tensor_mul(bvec[:], bvec[:], beps_sb[:])
                nc.vector.tensor_add(bvec[:], bvec[:], bmu_sb[:])

            # Whole sampled W shard, SBUF-resident for the full run.
            w_sb = wpool.tile([P, KT, O_SHARD], bf16, tag="w")

            def load_params(c):
                """DMA one chunk of mu/rho/eps (bf16) on the sync queue,
                interleaved with that chunk's x pieces; the scalar queue
                stays free so sampling ACTIVATEs run as soon as data lands."""
                CH = CHUNKS[c]
                kt0 = chunk_start[c]
                rows = slice(kt0 * P, (kt0 + CH) * P)
                mu = ppool.tile([P, CH, O_SHARD], bf16, tag="mu",
                                name=f"mu_{c}",
                                padded_shape=[P, CHMAX, O_SHARD])
                rho = ppool.tile([P, CH, O_SHARD], bf16, tag="rho",
                                 name=f"rho_{c}",
                                 padded_shape=[P, CHMAX, O_SHARD])
                eps = ppool.tile([P, CH, O_SHARD], bf16, tag="eps",
                                 name=f"eps_{c}",
                                 padded_shape=[P, CHMAX, O_SHARD])
                nc.sync.dma_start(
                    rho[:], w_rho_t[rows, :].rearrange("(j p) o -> p j o", p=P))
                nc.sync.dma_start(
                    mu[:], w_mu_t[rows, :].rearrange("(j p) o -> p j o", p=P))
                nc.sync.dma_start(
                    eps[:], eps_w_t[rows, :].rearrange("(j p) o -> p j o", p=P))
                return mu, rho, eps

            def sample_chunk(c, parts):
                """w = mu + softplus(rho) * eps for chunk c, into w_sb.
                DVE ops run in 2-ktile pieces so W slices unlock for the PE
                incrementally instead of all at the chunk boundary."""
                mu, rho, eps = parts
                CH = CHUNKS[c]
                kt0 = chunk_start[c]
                s = spool.tile([P, CH, O_SHARD], f32, tag="s", name=f"s_{c}",
                               padded_shape=[P, CHMAX, O_SHARD])
                nc.scalar.activation(s[:], rho[:], AF.Exp)
                nc.scalar.activation(s[:], s[:], AF.Ln, bias=1.0)
                for lo in range(0, CH, 2):
                    sl = slice(lo, min(lo + 2, CH))
                    nc.vector.tensor_mul(s[:, sl, :], s[:, sl, :],
                                         eps[:, sl, :])
                    nc.vector.tensor_add(w_sb[:, kt0 + sl.start:kt0 + sl.stop, :],
                                         s[:, sl, :], mu[:, sl, :])

            # x^T tiles: one SBUF tile per token tile, loaded in k-pieces.
            x_tiles = {}

            def x_tile(nt):
                if nt not in x_tiles:
                    x_tiles[nt] = xpool.tile([P, KT, N_TILE], bf16, tag="x",
                                             name=f"x_{nt}")
                return x_tiles[nt]

            def load_x_piece(nt, kt0, L):
                xt = x_tile(nt)
                nc.sync.dma_start(
                    xt[:, kt0:kt0 + L, :],
                    x_t[kt0 * P:(kt0 + L) * P,
                        nt * N_TILE:(nt + 1) * N_TILE]
                    .rearrange("(j p) n -> p j n", p=P))

            def drain(ps, nt, ms, engine):
                """y tile = psum + bias, then DMA out (SWDGE queue). Split
                across DVE and ACT so bank turnaround never serializes on
                one engine (ACT Copy computes in*1 + bias)."""
                ot = opool.tile([P, N_TILE], f32, tag="o", name=f"of_{nt}_{ms}")
                if engine == "v":
                    nc.vector.tensor_scalar_add(ot[:], ps[:], bvec[:, ms:ms + 1])
                else:
                    nc.scalar.activation(ot[:], ps[:], AF.Identity,
                                         bias=bvec[:, ms:ms + 1])
                nc.gpsimd.dma_start(
                    y_t[ms * P:(ms + 1) * P,
                        nt * N_TILE:(nt + 1) * N_TILE], ot[:])

            # ---- PE warmup: dummy matmuls while chunk-0 params stream ----
            # The HAM clock gate needs ~3.4us of sustained PE activity to
            # lift the PE from 1.2 to 2.4 GHz; burning that window on dummy
            # matmuls during the initial param DMA means the real matmuls
            # start warm (saves ~7us of half-rate matmuls + ~4us of head).
            # 26 dummies bridge the PE from the end of the framework preamble
            # (~7.5us) to first-chunk readiness (~14-15us) with no idle gap:
            # ~10 run at the cold clock, the rest warm, and the real matmuls
            # then start at full rate (an idle gap >3.4us would re-throttle).
            dum = bpool.tile([P, N_TILE], bf16, tag="dum")
            nc.vector.memset(dum[:], 0.0)
            psd = psum.tile([P, N_TILE], f32, tag="ps", name="ps_warm")
            for i in range(33):
                nc.tensor.matmul(psd[:], lhsT=dum[:, :P], rhs=dum[:],
                                 start=True, stop=True)

            # ---- phase A: token tiles 0-1, pipelined with param sampling.
            # Params/x prefetch TWO chunks ahead: the DMA issue+transfer+
            # sampling latency (~10us) exceeds one chunk's PE time (~7us),
            # so one-ahead prefetch stalled the PE ~2.5us at every chunk
            # boundary.
            nchunks = len(CHUNKS)
            parts = [None] * nchunks
            for c in range(min(2, nchunks)):
                parts[c] = load_params(c)
                load_x_piece(0, chunk_start[c], CHUNKS[c])
                load_x_piece(1, chunk_start[c], CHUNKS[c])
            sample_chunk(0, parts[0])
            compute_bias()

            # Matmuls are emitted in weight-paired order (same lhsT drives
            # the two token tiles back-to-back) so the weight load amortizes
            # over 2 matmuls.
            ps_a = [psum.tile([P, N_TILE], f32, tag="ps", name=f"psA_{i}")
                    for i in range(8)]
            for c, CH in enumerate(CHUNKS):
                kt0 = chunk_start[c]
                if c + 2 < nchunks:
                    parts[c + 2] = load_params(c + 2)
                    load_x_piece(0, chunk_start[c + 2], CHUNKS[c + 2])
                    load_x_piece(1, chunk_start[c + 2], CHUNKS[c + 2])
                if c + 1 < nchunks:
                    sample_chunk(c + 1, parts[c + 1])
                x01 = (x_tile(0), x_tile(1))
                for kt in range(kt0, kt0 + CH):
                    for ms in range(MS):
                        for nt in range(2):
                            nc.tensor.matmul(
                                ps_a[nt * MS + ms][:],
                                lhsT=w_sb[:, kt, ms * P:(ms + 1) * P],
                                rhs=x01[nt][:, kt, :],
                                start=(kt == 0),
                                stop=(kt == KT - 1),
                            )
            for nt in range(2):
                for ms in range(MS):
                    drain(ps_a[nt * MS + ms], nt, ms, "v" if ms % 2 else "a")

            # ---- phase B: token-tile pairs (2,3), (4,5), (6,7) ----
            # Each pair runs in two half-output stages (4 PSUM banks each):
            # stage 0 covers out-subtiles 0-1, stage 1 covers 2-3. Stage s+1
            # of one pair overlaps the drains of stage s, and the next pair's
            # stage 0 overlaps this pair's stage 1 — the PE never waits on
            # bank turnaround.
            PIECE = 8
            for nt in range(2, NT):
                for kp in range(0, KT, PIECE):
                    load_x_piece(nt, kp, PIECE)
            for a in range(2, NT, 2):
                b = a + 1
                xa, xb = x_tile(a), x_tile(b)
                for s in range(2):
                    mss = (2 * s, 2 * s + 1)
                    ps = {(nt, ms): psum.tile([P, N_TILE], f32, tag="ps",
                                              name=f"ps_{nt}_{ms}")
                          for ms in mss for nt in (a, b)}
                    for kt in range(KT):
                        for ms in mss:
                            for nt, xt in ((a, xa), (b, xb)):
                                nc.tensor.matmul(
                                    ps[(nt, ms)][:],
                                    lhsT=w_sb[:, kt, ms * P:(ms + 1) * P],
                                    rhs=xt[:, kt, :],
                                    start=(kt == 0),
                                    stop=(kt == KT - 1),
                                )
                    for ms in mss:
                        for nt in (a, b):
                            drain(ps[(nt, ms)], nt, ms,
                                  "v" if (nt + ms) % 2 else "a")

    nc.compile()
    return nc


def _get_nc():
    if "nc" not in _CACHE:
        _CACHE["nc"] = _build_nc()
    return _CACHE["nc"]


def _to_bf16(a):
    """Round-to-nearest-even fp32 -> bf16 without ml_dtypes astype overhead."""
    import ml_dtypes
    u = np.ascontiguousarray(a, dtype=np.float32).view(np.uint32)
    r = ((u + np.uint32(0x7FFF) + ((u >> np.uint32(16)) & np.uint32(1)))
         >> np.uint32(16)).astype(np.uint16)
    return r.view(ml_dtypes.bfloat16)


def _in_maps(inputs):
    x = np.asarray(inputs["x"], dtype=np.float32)
    w_mu = np.asarray(inputs["w_mu"], dtype=np.float32)
    w_rho = np.asarray(inputs["w_rho"], dtype=np.float32)
    eps_w = np.asarray(inputs["eps_w"], dtype=np.float32)
    b_mu = np.asarray(inputs["b_mu"], dtype=np.float32)
    b_rho = np.asarray(inputs["b_rho"], dtype=np.float32)
    eps_b = np.asarray(inputs["eps_b"], dtype=np.float32)

    x_t = _to_bf16(x.T)
    maps = []
    for c in range(N_CORES):
        sl = slice(c * O_SHARD, (c + 1) * O_SHARD)
        maps.append({
            "x_t": x_t,
            "w_mu_t": _to_bf16(w_mu[sl].T),
            "w_rho_t": _to_bf16(w_rho[sl].T),
            "eps_w_t": _to_bf16(eps_w[sl].T),
            "b_mu": np.ascontiguousarray(b_mu[sl]),
            "b_rho": np.ascontiguousarray(b_rho[sl]),
            "eps_b": np.ascontiguousarray(eps_b[sl]),
        })
    return maps


def run(inputs, trace=False, **kwargs):
    """Run on hardware; returns (y [N_TOK, OUT_F], BassKernelResults)."""
    from concourse.bass_utils import run_bass_kernel_spmd

    nc = _get_nc()
    res = run_bass_kernel_spmd(nc, _in_maps(inputs), list(range(N_CORES)),
                               trace=trace, **kwargs)
    y_t = np.concatenate([r["y_t"] for r in res.results], axis=0)
    return np.ascontiguousarray(y_t.T), res


def kernel(**inputs) -> np.ndarray:
    y, _ = run(inputs, trace=False)
    return y


# revision 37
# speedup vs baseline: 1.0303x; 1.0303x over previous
"""BayesianLinear forward on 8 Trainium2 NeuronCores.

y = x @ W^T + b with W = w_mu + softplus(w_rho) * eps_w,
                     b = b_mu + softplus(b_rho) * eps_b.

Sharding: column-parallel (output features / 8). Each core samples its
weight shard on-chip and computes y^T[o_shard, :] = W_shard @ x^T.

Inputs stream in bf16 (the 2e-2 tolerance leaves ~10x margin): this
halves HBM traffic vs fp32 (44 MiB/core vs 88), which is what made the
fp32 version DMA-bound (91% DMA busy, PE starved and HAM-throttled).
Here the PE is the bottleneck: 1024 back-to-back 128x128x512 matmuls
(~213 ns each warm) ~= 219 us/core.

Structure: the whole sampled W shard stays resident in SBUF (32 KiB/
partition bf16) and each (token-tile, out-subtile) accumulates its full
k=4096 contraction directly in one PSUM bank (no SBUF accumulators).
Phase A pipelines param-DMA -> softplus sampling -> matmuls across token
tiles 0-1 (8 open PSUM groups) so the PE starts ~4 us in while params
stream; phase B runs token tiles 2-7 k-dense with x double-buffered.
"""

import numpy as np

# Problem shape (hardcoded per contest rules; kernel.py must be self-contained).
IN_F = 4096
OUT_F = 4096
N_TOK = 4096
N_CORES = 8
O_SHARD = OUT_F // N_CORES  # 512 output features per core

P = 128                     # SBUF partitions
KT = IN_F // P              # 32 contraction tiles
MS = O_SHARD // P           # 4 output-feature subtiles per core
N_TILE = 512                # moving-operand tile (fp32 PSUM bank limit)
NT = N_TOK // N_TILE        # 8 token tiles

# Param/sampling chunk sizes (k-tiles per chunk) for the phase-A pipeline.
# Uniform 4-ktile chunks: a 2-ktile chunk is only ~3.4us of PE work, not
# enough for the two-chunk-ahead prefetch to cover the ~10us param
# DMA+sampling latency.
CHUNKS = [2, 4, 4, 4, 4, 4, 4, 4, 2]
CHMAX = max(CHUNKS)
assert sum(CHUNKS) == KT

_CACHE = {}


def _pin_act_table(bacc, mybir):
    """Keep Exp and Ln only in the one ACT table that has both, so the
    compiler never inserts per-op table reloads (1.3 us each, and they sit
    on the weight-sampling critical path)."""
    if getattr(bacc.get_activation_tables, "_pinned", False):
        return
    orig = bacc.get_activation_tables
    EXP = mybir.ActivationFunctionType.Exp
    LN = mybir.ActivationFunctionType.Ln

    def pinned(arch):
        tables = orig(arch)
        for name, funcs in tables.items():
            if name != "natural_log_exp_and_others":
                funcs.discard(EXP)
                funcs.discard(LN)
        return tables

    pinned._pinned = True
    bacc.get_activation_tables = pinned


def _build_nc():
    import concourse.bass as bass  # noqa: F401
    from concourse import bacc, mybir
    from concourse.tile import TileContext

    _pin_act_table(bacc, mybir)

    f32 = mybir.dt.float32
    f32r = mybir.dt.float32r
    bf16 = mybir.dt.bfloat16
    AF = mybir.ActivationFunctionType

    nc = bacc.Bacc("TRN2", target_bir_lowering=False, debug=False,
                   num_devices=N_CORES)

    x_t = nc.dram_tensor("x_t", [IN_F, N_TOK], bf16, kind="ExternalInput")
    w_mu_t = nc.dram_tensor("w_mu_t", [IN_F, O_SHARD], bf16, kind="ExternalInput")
    w_rho_t = nc.dram_tensor("w_rho_t", [IN_F, O_SHARD], bf16, kind="ExternalInput")
    eps_w_t = nc.dram_tensor("eps_w_t", [IN_F, O_SHARD], bf16, kind="ExternalInput")
    b_mu = nc.dram_tensor("b_mu", [O_SHARD], f32, kind="ExternalInput")
    b_rho = nc.dram_tensor("b_rho", [O_SHARD], f32, kind="ExternalInput")
    eps_b = nc.dram_tensor("eps_b", [O_SHARD], f32, kind="ExternalInput")
    y_t = nc.dram_tensor("y_t", [O_SHARD, N_TOK], f32, kind="ExternalOutput")

    chunk_start = [sum(CHUNKS[:c]) for c in range(len(CHUNKS))]

    with TileContext(nc) as tc:
        with (
            tc.tile_pool(name="wpool", bufs=1) as wpool,
            tc.tile_pool(name="ppool", bufs=3) as ppool,
            tc.tile_pool(name="spool", bufs=2) as spool,
            tc.tile_pool(name="bpool", bufs=1) as bpool,
            tc.tile_pool(name="xpool", bufs=3) as xpool,
            tc.tile_pool(name="opool", bufs=3) as opool,
            tc.tile_pool(name="psum", bufs=8, space="PSUM") as psum,
        ):
            # ---- bias vector: b = b_mu + softplus(b_rho) * eps_b ----
            # laid out [P, MS]: partition p of output subtile ms holds
            # b[ms*128 + p].
            bmu_sb = bpool.tile([P, MS], f32, tag="bmu")
            brho_sb = bpool.tile([P, MS], f32, tag="brho")
            beps_sb = bpool.tile([P, MS], f32, tag="beps")
            bvec = bpool.tile([P, MS], f32, tag="bvec")

            def compute_bias():
                nc.scalar.dma_start(bmu_sb[:],
                                    b_mu.rearrange("(s p) -> p s", p=P))
                nc.scalar.dma_start(brho_sb[:],
                                    b_rho.rearrange("(s p) -> p s", p=P))
                nc.scalar.dma_start(beps_sb[:],
                                    eps_b.rearrange("(s p) -> p s", p=P))
                # softplus(r) = ln(1 + exp(r)); Exp/Ln share one ACT table
                # and the Ln's +1 folds into the activation bias operand.
                nc.scalar.activation(bvec[:], brho_sb[:], AF.Exp)
                nc.scalar.activation(bvec[:], bvec[:], AF.Ln, bias=1.0)
                nc.vector.tensor_mul(bvec[:], bvec[:], beps_sb[:])
                nc.vector.tensor_add(bvec[:], bvec[:], bmu_sb[:])

            # Whole sampled W shard, SBUF-resident for the full run.
            w_sb = wpool.tile([P, KT, O_SHARD], bf16, tag="w")

            def load_params(c):
                """DMA one chunk of mu/rho/eps (bf16) on the sync queue,
                interleaved with that chunk's x pieces; the scalar queue
                stays free so sampling ACTIVATEs run as soon as data lands.
                Chunk 0 is the kernel's critical path: its rho goes out
                first and alone on sync, mu/eps ride the idle gpsimd queue,
                skipping ~1.5us of serial descriptor-gen."""
                CH = CHUNKS[c]
                kt0 = chunk_start[c]
                rows = slice(kt0 * P, (kt0 + CH) * P)
                mu = ppool.tile([P, CH, O_SHARD], bf16, tag="mu",
                                name=f"mu_{c}",
                                padded_shape=[P, CHMAX, O_SHARD])
                rho = ppool.tile([P, CH, O_SHARD], bf16, tag="rho",
                                 name=f"rho_{c}",
                                 padded_shape=[P, CHMAX, O_SHARD])
                eps = ppool.tile([P, CH, O_SHARD], bf16, tag="eps",
                                 name=f"eps_{c}",
                                 padded_shape=[P, CHMAX, O_SHARD])
                other_eng = nc.gpsimd if c == 0 else nc.sync
                nc.sync.dma_start(
                    rho[:], w_rho_t[rows, :].rearrange("(j p) o -> p j o", p=P))
                other_eng.dma_start(
                    mu[:], w_mu_t[rows, :].rearrange("(j p) o -> p j o", p=P))
                other_eng.dma_start(
                    eps[:], eps_w_t[rows, :].rearrange("(j p) o -> p j o", p=P))
                return mu, rho, eps

            def sample_chunk(c, parts):
                """w = mu + softplus(rho) * eps for chunk c, into w_sb.
                The exp->ln->mul->add chain is fully serial per element, so
                it runs in small k-tile pieces with ACT and DVE interleaved:
                piece 0's DVE ops overlap piece 1's ACT ops and the first W
                slices unlock for the PE after one piece's latency instead
                of the whole chunk's (~6.5us for a 4-ktile chunk)."""
                mu, rho, eps = parts
                CH = CHUNKS[c]
                kt0 = chunk_start[c]
                step = 1 if c == 0 else 2
                s = spool.tile([P, CH, O_SHARD], f32, tag="s", name=f"s_{c}",
                               padded_shape=[P, CHMAX, O_SHARD])
                for lo in range(0, CH, step):
                    sl = slice(lo, min(lo + step, CH))
                    nc.scalar.activation(s[:, sl, :], rho[:, sl, :], AF.Exp)
                    nc.scalar.activation(s[:, sl, :], s[:, sl, :], AF.Ln,
                                         bias=1.0)
                    nc.vector.tensor_mul(s[:, sl, :], s[:, sl, :],
                                         eps[:, sl, :])
                    nc.vector.tensor_add(w_sb[:, kt0 + sl.start:kt0 + sl.stop, :],
                                         s[:, sl, :], mu[:, sl, :])

            # x^T tiles: one SBUF tile per token tile, loaded in k-pieces.
            # The second token tile of each phase-B pair (nt 3/5/7) splits
            # its first 8 k-tiles into a small separate "head" tile: the
            # main xpool slot only frees when the previous pair retires, so
            # without the head the pair's opening matmuls wait ~2us on DMA.
            x_tiles = {}

            def x_tile(nt):
                if nt not in x_tiles:
                    x_tiles[nt] = xpool.tile([P, KT, N_TILE], bf16, tag="x",
                                             name=f"x_{nt}")
                return x_tiles[nt]

            def x_rhs(nt, kt):
                return x_tiles[nt][:, kt, :]

            def load_x_piece(nt, kt0, L):
                xt = x_tile(nt)
                nc.sync.dma_start(
                    xt[:, kt0:kt0 + L, :],
                    x_t[kt0 * P:(kt0 + L) * P,
                        nt * N_TILE:(nt + 1) * N_TILE]
                    .rearrange("(j p) n -> p j n", p=P))

            def drain(ps, nt, ms, engine):
                """y tile = psum + bias, then DMA out (SWDGE queue). Split
                across DVE and ACT so bank turnaround never serializes on
                one engine (ACT Copy computes in*1 + bias)."""
                ot = opool.tile([P, N_TILE], f32, tag="o", name=f"of_{nt}_{ms}")
                if engine == "v":
                    nc.vector.tensor_scalar_add(ot[:], ps[:], bvec[:, ms:ms + 1])
                else:
                    nc.scalar.activation(ot[:], ps[:], AF.Identity,
                                         bias=bvec[:, ms:ms + 1])
                nc.gpsimd.dma_start(
                    y_t[ms * P:(ms + 1) * P,
                        nt * N_TILE:(nt + 1) * N_TILE], ot[:])

            # ---- PE warmup: dummy matmuls while chunk-0 params stream ----
            # The HAM clock gate needs ~3.4us of sustained PE activity to
            # lift the PE from 1.2 to 2.4 GHz; burning that window on dummy
            # matmuls during the initial param DMA means the real matmuls
            # start warm (saves ~7us of half-rate matmuls + ~4us of head).
            # 26 dummies bridge the PE from the end of the framework preamble
            # (~7.5us) to first-chunk readiness (~14-15us) with no idle gap:
            # ~10 run at the cold clock, the rest warm, and the real matmuls
            # then start at full rate (an idle gap >3.4us would re-throttle).
            dum = bpool.tile([P, N_TILE], bf16, tag="dum")
            nc.vector.memset(dum[:], 0.0)
            psd = psum.tile([P, N_TILE], f32, tag="ps", name="ps_warm")
            for i in range(18):
                nc.tensor.matmul(psd[:], lhsT=dum[:, :P], rhs=dum[:],
                                 start=True, stop=True)

            # ---- phase A: token tiles 0-1, pipelined with param sampling.
            # Params/x prefetch TWO chunks ahead: the DMA issue+transfer+
            # sampling latency (~10us) exceeds one chunk's PE time (~7us),
            # so one-ahead prefetch stalled the PE ~2.5us at every chunk
            # boundary.
            nchunks = len(CHUNKS)
            parts = [None] * nchunks
            for c in range(min(2, nchunks)):
                parts[c] = load_params(c)
                load_x_piece(0, chunk_start[c], CHUNKS[c])
                load_x_piece(1, chunk_start[c], CHUNKS[c])
            sample_chunk(0, parts[0])

            # Matmuls are emitted in weight-paired order (same lhsT drives
            # the two token tiles back-to-back) so the weight load amortizes
            # over 2 matmuls.
            ps_a = [psum.tile([P, N_TILE], f32, tag="ps", name=f"psA_{i}")
                    for i in range(8)]
            for c, CH in enumerate(CHUNKS):
                kt0 = chunk_start[c]
                if c + 2 < nchunks:
                    parts[c + 2] = load_params(c + 2)
                    load_x_piece(0, chunk_start[c + 2], CHUNKS[c + 2])
                    load_x_piece(1, chunk_start[c + 2], CHUNKS[c + 2])
                if c + 1 < nchunks:
                    sample_chunk(c + 1, parts[c + 1])
                x01 = (x_tile(0), x_tile(1))
                for kt in range(kt0, kt0 + CH):
                    for ms in range(MS):
                        for nt in range(2):
                            nc.tensor.matmul(
                                ps_a[nt * MS + ms][:],
                                lhsT=w_sb[:, kt, ms * P:(ms + 1) * P],
                                rhs=x01[nt][:, kt, :],
                                start=(kt == 0),
                                stop=(kt == KT - 1),
                            )
            # Bias is only needed by the drains below; emitting it here keeps
            # its ACT/DVE ops off the chunk-0 sampling critical path.
            compute_bias()
            for nt in range(2):
                for ms in range(MS):
                    drain(ps_a[nt * MS + ms], nt, ms, "v" if ms % 2 else "a")

            # ---- phase B: token-tile pairs (2,3), (4,5), (6,7) ----
            # Each pair runs in two half-output stages (4 PSUM banks each):
            # stage 0 covers out-subtiles 0-1, stage 1 covers 2-3. Stage s+1
            # of one pair overlaps the drains of stage s, and the next pair's
            # stage 0 overlaps this pair's stage 1 — the PE never waits on
            # bank turnaround.
            PIECE = 8
            for nt in range(2, NT):
                for kp in range(0, KT, PIECE):
                    load_x_piece(nt, kp, PIECE)
            for a in range(2, NT, 2):
                b = a + 1
                x_tile(a), x_tile(b)
                for s in range(2):
                    mss = (2 * s, 2 * s + 1)
                    ps = {(nt, ms): psum.tile([P, N_TILE], f32, tag="ps",
                                              name=f"ps_{nt}_{ms}")
                          for ms in mss for nt in (a, b)}
                    for kt in range(KT):
                        for ms in mss:
                            for nt in (a, b):
                                nc.tensor.matmul(
                                    ps[(nt, ms)][:],
                                    lhsT=w_sb[:, kt, ms * P:(ms + 1) * P],
                                    rhs=x_rhs(nt, kt),
                                    start=(kt == 0),
                                    stop=(kt == KT - 1),
                                )
                    for ms in mss:
                        for nt in (a, b):
                            drain(ps[(nt, ms)], nt, ms,
                                  "v" if (nt + ms) % 2 else "a")

    nc.compile()
    return nc


def _get_nc():
    if "nc" not in _CACHE:
        _CACHE["nc"] = _build_nc()
    return _CACHE["nc"]


def _to_bf16(a):
    """Round-to-nearest-even fp32 -> bf16 without ml_dtypes astype overhead."""
    import ml_dtypes
    u = np.ascontiguousarray(a, dtype=np.float32).view(np.uint32)
    r = ((u + np.uint32(0x7FFF) + ((u >> np.uint32(16)) & np.uint32(1)))
         >> np.uint32(16)).astype(np.uint16)
    return r.view(ml_dtypes.bfloat16)


def _in_maps(inputs):
    x = np.asarray(inputs["x"], dtype=np.float32)
    w_mu = np.asarray(inputs["w_mu"], dtype=np.float32)
    w_rho = np.asarray(inputs["w_rho"], dtype=np.float32)
    eps_w = np.asarray(inputs["eps_w"], dtype=np.float32)
    b_mu = np.asarray(inputs["b_mu"], dtype=np.float32)
    b_rho = np.asarray(inputs["b_rho"], dtype=np.float32)
    eps_b = np.asarray(inputs["eps_b"], dtype=np.float32)

    x_t = _to_bf16(x.T)
    maps = []
    for c in range(N_CORES):
        sl = slice(c * O_SHARD, (c + 1) * O_SHARD)
        maps.append({
            "x_t": x_t,
            "w_mu_t": _to_bf16(w_mu[sl].T),
            "w_rho_t": _to_bf16(w_rho[sl].T),
            "eps_w_t": _to_bf16(eps_w[sl].T),
            "b_mu": np.ascontiguousarray(b_mu[sl]),
            "b_rho": np.ascontiguousarray(b_rho[sl]),
            "eps_b": np.ascontiguousarray(eps_b[sl]),
        })
    return maps


def run(inputs, trace=False, **kwargs):
    """Run on hardware; returns (y [N_TOK, OUT_F], BassKernelResults)."""
    from concourse.bass_utils import run_bass_kernel_spmd

    nc = _get_nc()
    res = run_bass_kernel_spmd(nc, _in_maps(inputs), list(range(N_CORES)),
                               trace=trace, **kwargs)
    y_t = np.concatenate([r["y_t"] for r in res.results], axis=0)
    return np.ascontiguousarray(y_t.T), res


def kernel(**inputs) -> np.ndarray:
    y, _ = run(inputs, trace=False)
    return y
